# revision 3
# baseline (speedup 1.0000x reference)
"""Self-contained Trainium2 Bass kernel for the LRU (linear recurrent unit) problem.

reference semantics (T=32768, M=H=512):
    lam   = exp(-exp(nu_log) + 1j*exp(theta_log))            # [H]
    Bn    = (B_re + 1j*B_im) * exp(gamma_log)[:,None]        # [H,M]
    Bu    = x @ Bn.T                                         # [T,H] complex
    h_t   = lam * (1-start[t]) * h_{t-1} + Bu_t   (h_{-1} = state)
    out   = Re(h @ C.T) + x*D                                # [T,M]
    returns (h[None,-1] complex64, out float32)

HW mapping: shard time across 8 cores (4096 steps each + V=128 overlap; episode
resets inside the overlap window make a zero initial state exact — verified on
CPU at runtime).  Within a core, rotate each channel into the frame
g_tau = e^{-i*theta*tau} h_tau so the recurrence multiplier becomes the real
scalar r*keep, which maps onto per-partition-lane `tensor_tensor_scan` on the
vector engine.  Matmuls (B projection, C projection, diag(D) skip connection)
run on the tensor engine.
"""

import numpy as np

T, M, H = 32768, 512, 512
NCORES = 8
SH = T // NCORES        # 4096 timesteps per core
V = 128                 # overlap margin (multiple of 128)
S = SH + V              # 4224 processed steps per core
TT = 512                # time tile
G = 4                   # channel groups of 128

_CACHE: dict = {}


def _build_bass():
    import concourse.tile as tile
    import concourse.mybir as mybir
    from concourse import bacc

    dtf = mybir.dt.float32
    AF = mybir.ActivationFunctionType
    OP = mybir.AluOpType

    nc = bacc.Bacc()
    xT = nc.dram_tensor("xT", [M, S], dtf, kind="ExternalInput")
    av = nc.dram_tensor("av", [H, S], dtf, kind="ExternalInput")
    ct = nc.dram_tensor("ct", [H, S], dtf, kind="ExternalInput")
    st = nc.dram_tensor("st", [H, S], dtf, kind="ExternalInput")
    wbre = nc.dram_tensor("wbre", [M, H], dtf, kind="ExternalInput")
    wbim = nc.dram_tensor("wbim", [M, H], dtf, kind="ExternalInput")
    cre = nc.dram_tensor("cre", [H, M], dtf, kind="ExternalInput")
    cimn = nc.dram_tensor("cimn", [H, M], dtf, kind="ExternalInput")
    dd = nc.dram_tensor("dd", [M, M], dtf, kind="ExternalInput")
    out = nc.dram_tensor("out", [SH, M], dtf, kind="ExternalOutput")
    hlast = nc.dram_tensor("hlast", [2 * G, 128], dtf, kind="ExternalOutput")

    ntiles = (S + TT - 1) // TT

    with tile.TileContext(nc) as tc:
        with (
            tc.tile_pool(name="singles", bufs=1) as singles,
            tc.tile_pool(name="stream", bufs=2) as stream,
            tc.tile_pool(name="work", bufs=2) as work,
            tc.tile_pool(name="hkeep", bufs=2) as hkeep,
            tc.tile_pool(name="outp", bufs=3) as outp,
            tc.tile_pool(name="psum", bufs=2, space="PSUM") as psum,
        ):
            wbre_sb = singles.tile([128, G, H], dtf)
            nc.sync.dma_start(out=wbre_sb, in_=wbre.rearrange("(k p) h -> p k h", p=128))
            wbim_sb = singles.tile([128, G, H], dtf)
            nc.sync.dma_start(out=wbim_sb, in_=wbim.rearrange("(k p) h -> p k h", p=128))
            cre_sb = singles.tile([128, G, M], dtf)
            nc.sync.dma_start(out=cre_sb, in_=cre.rearrange("(g p) m -> p g m", p=128))
            cimn_sb = singles.tile([128, G, M], dtf)
            nc.sync.dma_start(out=cimn_sb, in_=cimn.rearrange("(g p) m -> p g m", p=128))
            dd_sb = singles.tile([128, G, M], dtf)
            nc.sync.dma_start(out=dd_sb, in_=dd.rearrange("(k p) m -> p k m", p=128))
            carry = singles.tile([128, 2 * G], dtf)
            nc.vector.memset(carry, 0.0)

            for i in range(ntiles):
                t0 = i * TT
                Ti = min(TT, S - t0)
                nj = Ti // 128

                xt = stream.tile([128, G, TT], dtf, tag="xt")
                nc.sync.dma_start(
                    out=xt[:, :, :Ti],
                    in_=xT[:, t0 : t0 + Ti].rearrange("(k p) t -> p k t", p=128),
                )
                at = stream.tile([128, G, TT], dtf, tag="at")
                nc.sync.dma_start(
                    out=at[:, :, :Ti],
                    in_=av[:, t0 : t0 + Ti].rearrange("(g p) t -> p g t", p=128),
                )
                ctt = stream.tile([128, G, TT], dtf, tag="ctt")
                nc.sync.dma_start(
                    out=ctt[:, :, :Ti],
                    in_=ct[:, t0 : t0 + Ti].rearrange("(g p) t -> p g t", p=128),
                )
                stt = stream.tile([128, G, TT], dtf, tag="stt")
                nc.sync.dma_start(
                    out=stt[:, :, :Ti],
                    in_=st[:, t0 : t0 + Ti].rearrange("(g p) t -> p g t", p=128),
                )

                hres, hims = [], []
                for g in range(G):
                    gs = slice(g * 128, (g + 1) * 128)
                    # Bu = x @ Bn.T  (re/im), laid out [h-lane, t]
                    bre = psum.tile([128, TT], dtf, tag="bre")
                    for k in range(G):
                        nc.tensor.matmul(
                            bre[:, :Ti], wbre_sb[:, k, gs], xt[:, k, :Ti],
                            start=(k == 0), stop=(k == G - 1),
                        )
                    bim = psum.tile([128, TT], dtf, tag="bim")
                    for k in range(G):
                        nc.tensor.matmul(
                            bim[:, :Ti], wbim_sb[:, k, gs], xt[:, k, :Ti],
                            start=(k == 0), stop=(k == G - 1),
                        )

                    ctg = ctt[:, g, :Ti]
                    stg = stt[:, g, :Ti]

                    # g_in = e^{-i theta tau} * Bu
                    ginr = work.tile([128, TT], dtf, tag="ginr")
                    gini = work.tile([128, TT], dtf, tag="gini")
                    tmp1 = work.tile([128, TT], dtf, tag="tmp1")
                    tmp2 = work.tile([128, TT], dtf, tag="tmp2")
                    nc.vector.tensor_mul(ginr[:, :Ti], ctg, bre[:, :Ti])
                    nc.vector.tensor_mul(tmp1[:, :Ti], stg, bim[:, :Ti])
                    nc.vector.tensor_add(ginr[:, :Ti], ginr[:, :Ti], tmp1[:, :Ti])
                    nc.vector.tensor_mul(gini[:, :Ti], ctg, bim[:, :Ti])
                    nc.vector.tensor_mul(tmp2[:, :Ti], stg, bre[:, :Ti])
                    nc.vector.tensor_sub(gini[:, :Ti], gini[:, :Ti], tmp2[:, :Ti])

                    # g_tau = a*g_{tau-1} + g_in   (real recurrence, per lane)
                    gsr = work.tile([128, TT], dtf, tag="gsr")
                    gsi = work.tile([128, TT], dtf, tag="gsi")
                    nc.vector.tensor_tensor_scan(
                        gsr[:, :Ti], at[:, g, :Ti], ginr[:, :Ti],
                        carry[:, 2 * g : 2 * g + 1], op0=OP.mult, op1=OP.add,
                    )
                    nc.vector.tensor_copy(carry[:, 2 * g : 2 * g + 1], gsr[:, Ti - 1 : Ti])
                    nc.vector.tensor_tensor_scan(
                        gsi[:, :Ti], at[:, g, :Ti], gini[:, :Ti],
                        carry[:, 2 * g + 1 : 2 * g + 2], op0=OP.mult, op1=OP.add,
                    )
                    nc.vector.tensor_copy(carry[:, 2 * g + 1 : 2 * g + 2], gsi[:, Ti - 1 : Ti])

                    # h = e^{+i theta tau} * g
                    hre = hkeep.tile([128, TT], dtf, tag=f"hre{g}")
                    him = hkeep.tile([128, TT], dtf, tag=f"him{g}")
                    tmp3 = work.tile([128, TT], dtf, tag="tmp3")
                    tmp4 = work.tile([128, TT], dtf, tag="tmp4")
                    nc.vector.tensor_mul(hre[:, :Ti], ctg, gsr[:, :Ti])
                    nc.vector.tensor_mul(tmp3[:, :Ti], stg, gsi[:, :Ti])
                    nc.vector.tensor_sub(hre[:, :Ti], hre[:, :Ti], tmp3[:, :Ti])
                    nc.vector.tensor_mul(him[:, :Ti], ctg, gsi[:, :Ti])
                    nc.vector.tensor_mul(tmp4[:, :Ti], stg, gsr[:, :Ti])
                    nc.vector.tensor_add(him[:, :Ti], him[:, :Ti], tmp4[:, :Ti])
                    hres.append(hre)
                    hims.append(him)

                    if i == ntiles - 1:
                        nc.sync.dma_start(out=hlast[g, :], in_=hre[:, Ti - 1 : Ti])
                        nc.sync.dma_start(out=hlast[G + g, :], in_=him[:, Ti - 1 : Ti])

                # out[t,m] = sum_h C_re[m,h] h_re - C_im[m,h] h_im + sum_m' x[t,m'] dd[m',m]
                for j in range(nj):
                    glob0 = t0 + j * 128
                    js = slice(j * 128, (j + 1) * 128)
                    ops = psum.tile([128, M], dtf, tag="ops")
                    for g in range(G):
                        nc.tensor.matmul(
                            ops, hres[g][:, js], cre_sb[:, g, :],
                            start=(g == 0), stop=False,
                        )
                        nc.tensor.matmul(
                            ops, hims[g][:, js], cimn_sb[:, g, :],
                            start=False, stop=False,
                        )
                    for k in range(G):
                        nc.tensor.matmul(
                            ops, xt[:, k, js], dd_sb[:, k, :],
                            start=False, stop=(k == G - 1),
                        )
                    if glob0 >= V:
                        osb = outp.tile([128, M], dtf, tag="osb")
                        nc.scalar.activation(osb, ops, AF.Copy)
                        nc.sync.dma_start(out=out[glob0 - V : glob0 - V + 128, :], in_=osb)
    nc.finalize()
    return nc


def _prep(state, x, start, theta_log, nu_log, gamma_log, B_re, B_im, C_re, C_im, D):
    theta = np.exp(np.asarray(theta_log, np.float64))
    r = np.exp(-np.exp(np.asarray(nu_log, np.float64)))          # [H]
    gamma = np.exp(np.asarray(gamma_log, np.float64))
    wbre = np.ascontiguousarray((np.asarray(B_re, np.float64) * gamma[:, None]).T.astype(np.float32))
    wbim = np.ascontiguousarray((np.asarray(B_im, np.float64) * gamma[:, None]).T.astype(np.float32))
    cre = np.ascontiguousarray(np.asarray(C_re, np.float32).T)
    cimn = np.ascontiguousarray((-np.asarray(C_im, np.float32)).T)
    dd = np.ascontiguousarray(np.diag(np.asarray(D, np.float32)))

    tau = np.arange(S, dtype=np.float64)
    ang = theta[:, None] * tau[None, :]
    ctab = np.ascontiguousarray(np.cos(ang).astype(np.float32))
    stab = np.ascontiguousarray(np.sin(ang).astype(np.float32))

    startb = np.asarray(start, bool)
    keep = (1.0 - startb.astype(np.float32))
    rf = r.astype(np.float32)
    x32 = np.asarray(x, np.float32)

    in_maps = []
    for c in range(NCORES):
        lo = c * SH - V
        if c == 0:
            xs = np.concatenate([np.zeros((V, M), np.float32), x32[:SH]], axis=0)
            ks = np.concatenate([np.ones(V, np.float32), keep[:SH]])
        else:
            xs = x32[lo : lo + S]
            ks = keep[lo : lo + S]
        in_maps.append(
            dict(
                xT=np.ascontiguousarray(xs.T),
                av=np.ascontiguousarray(rf[:, None] * ks[None, :]),
                ct=ctab, st=stab, wbre=wbre, wbim=wbim,
                cre=cre, cimn=cimn, dd=dd,
            )
        )
    return in_maps, theta, r, startb


def _boundary_ok(startb):
    return all(startb[c * SH - V : c * SH].any() for c in range(1, NCORES))


def kernel(state, x, start, theta_log, nu_log, gamma_log, B_re, B_im, C_re, C_im, D,
           _want_results=False):
    from concourse.bass_utils import run_bass_kernel_spmd

    in_maps, theta, r, startb = _prep(
        state, x, start, theta_log, nu_log, gamma_log, B_re, B_im, C_re, C_im, D
    )
    assert _boundary_ok(startb), "no episode reset inside an overlap window; V too small"

    if "nc" not in _CACHE:
        _CACHE["nc"] = _build_bass()
    nc = _CACHE["nc"]

    res = run_bass_kernel_spmd(nc, in_maps, list(range(NCORES))).results
    out = np.concatenate([res[c]["out"] for c in range(NCORES)], axis=0)
    hl = res[NCORES - 1]["hlast"]
    hfin = (hl[:G].reshape(H) + 1j * hl[G:].reshape(H)).astype(np.complex64)[None, :]

    # exact correction for a nonzero initial state (zero for the spec inputs):
    s0 = np.asarray(state, np.float64)[0]
    if np.any(s0 != 0):
        lam = r * np.exp(1j * theta)
        C = np.asarray(C_re, np.float64) + 1j * np.asarray(C_im, np.float64)
        fr = int(startb.argmax()) if startb.any() else T
        cum = s0.astype(np.complex128)
        for t in range(fr):
            cum = cum * lam
            out[t] += (C @ cum).real.astype(np.float32)
        if fr == T:
            hfin = (hfin[0] + cum.astype(np.complex64))[None, :]
    return hfin, out


# revision 5
# speedup vs baseline: 1.1774x; 1.1774x over previous
"""Self-contained Trainium2 Bass kernel for the LRU (linear recurrent unit) problem.

reference semantics (T=32768, M=H=512):
    lam   = exp(-exp(nu_log) + 1j*exp(theta_log))            # [H]
    Bn    = (B_re + 1j*B_im) * exp(gamma_log)[:,None]        # [H,M]
    Bu    = x @ Bn.T                                         # [T,H] complex
    h_t   = lam * (1-start[t]) * h_{t-1} + Bu_t   (h_{-1} = state)
    out   = Re(h @ C.T) + x*D                                # [T,M]
    returns (h[None,-1] complex64, out float32)

HW mapping: shard time across 8 cores (4096 steps each + V=128 overlap; episode
resets inside the overlap window make a zero initial state exact — verified on
CPU at runtime).  Within a core, rotate each channel into the frame
g_tau = e^{-i*theta*tau} h_tau so the recurrence multiplier becomes the real
scalar r*keep, which maps onto per-partition-lane `tensor_tensor_scan` on the
vector engine.  Matmuls (B projection, C projection, diag(D) skip connection)
run on the tensor engine.
"""

import numpy as np

T, M, H = 32768, 512, 512
NCORES = 8
SH = T // NCORES        # 4096 timesteps per core
V = 128                 # overlap margin (multiple of 128)
S = SH + V              # 4224 processed steps per core
TT = 512                # time tile
G = 4                   # channel groups of 128

_CACHE: dict = {}


def _build_bass():
    import concourse.tile as tile
    import concourse.mybir as mybir
    from concourse import bacc

    dtf = mybir.dt.float32
    AF = mybir.ActivationFunctionType
    OP = mybir.AluOpType

    nc = bacc.Bacc()
    xT = nc.dram_tensor("xT", [M, S], dtf, kind="ExternalInput")
    av = nc.dram_tensor("av", [H, S], dtf, kind="ExternalInput")
    ct = nc.dram_tensor("ct", [H, S], dtf, kind="ExternalInput")
    st = nc.dram_tensor("st", [H, S], dtf, kind="ExternalInput")
    wbre = nc.dram_tensor("wbre", [M, H], dtf, kind="ExternalInput")
    wbim = nc.dram_tensor("wbim", [M, H], dtf, kind="ExternalInput")
    cre = nc.dram_tensor("cre", [H, M], dtf, kind="ExternalInput")
    cimn = nc.dram_tensor("cimn", [H, M], dtf, kind="ExternalInput")
    dd = nc.dram_tensor("dd", [M, M], dtf, kind="ExternalInput")
    out = nc.dram_tensor("out", [SH, M], dtf, kind="ExternalOutput")
    hlast = nc.dram_tensor("hlast", [2 * G, 128], dtf, kind="ExternalOutput")

    ntiles = (S + TT - 1) // TT

    with tile.TileContext(nc) as tc:
        with (
            tc.tile_pool(name="singles", bufs=1) as singles,
            tc.tile_pool(name="stream", bufs=2) as stream,
            tc.tile_pool(name="work", bufs=2) as work,
            tc.tile_pool(name="hkeep", bufs=2) as hkeep,
            tc.tile_pool(name="outp", bufs=3) as outp,
            tc.tile_pool(name="psum", bufs=2, space="PSUM") as psum,
        ):
            wbre_sb = singles.tile([128, G, H], dtf)
            nc.sync.dma_start(out=wbre_sb, in_=wbre.rearrange("(k p) h -> p k h", p=128))
            wbim_sb = singles.tile([128, G, H], dtf)
            nc.sync.dma_start(out=wbim_sb, in_=wbim.rearrange("(k p) h -> p k h", p=128))
            cre_sb = singles.tile([128, G, M], dtf)
            nc.sync.dma_start(out=cre_sb, in_=cre.rearrange("(g p) m -> p g m", p=128))
            cimn_sb = singles.tile([128, G, M], dtf)
            nc.sync.dma_start(out=cimn_sb, in_=cimn.rearrange("(g p) m -> p g m", p=128))
            dd_sb = singles.tile([128, G, M], dtf)
            nc.sync.dma_start(out=dd_sb, in_=dd.rearrange("(k p) m -> p k m", p=128))
            carry = singles.tile([128, 2 * G], dtf)
            nc.vector.memset(carry, 0.0)

            for i in range(ntiles):
                t0 = i * TT
                Ti = min(TT, S - t0)
                nj = Ti // 128

                xt = stream.tile([128, G, TT], dtf, tag="xt")
                nc.sync.dma_start(
                    out=xt[:, :, :Ti],
                    in_=xT[:, t0 : t0 + Ti].rearrange("(k p) t -> p k t", p=128),
                )
                at = stream.tile([128, G, TT], dtf, tag="at")
                nc.sync.dma_start(
                    out=at[:, :, :Ti],
                    in_=av[:, t0 : t0 + Ti].rearrange("(g p) t -> p g t", p=128),
                )
                ctt = stream.tile([128, G, TT], dtf, tag="ctt")
                nc.sync.dma_start(
                    out=ctt[:, :, :Ti],
                    in_=ct[:, t0 : t0 + Ti].rearrange("(g p) t -> p g t", p=128),
                )
                stt = stream.tile([128, G, TT], dtf, tag="stt")
                nc.sync.dma_start(
                    out=stt[:, :, :Ti],
                    in_=st[:, t0 : t0 + Ti].rearrange("(g p) t -> p g t", p=128),
                )

                hres, hims = [], []
                for g in range(G):
                    gs = slice(g * 128, (g + 1) * 128)
                    # Bu = x @ Bn.T  (re/im), laid out [h-lane, t]
                    bre = psum.tile([128, TT], dtf, tag="bre")
                    for k in range(G):
                        nc.tensor.matmul(
                            bre[:, :Ti], wbre_sb[:, k, gs], xt[:, k, :Ti],
                            start=(k == 0), stop=(k == G - 1),
                        )
                    bim = psum.tile([128, TT], dtf, tag="bim")
                    for k in range(G):
                        nc.tensor.matmul(
                            bim[:, :Ti], wbim_sb[:, k, gs], xt[:, k, :Ti],
                            start=(k == 0), stop=(k == G - 1),
                        )

                    ctg = ctt[:, g, :Ti]
                    stg = stt[:, g, :Ti]

                    # g_in = e^{-i theta tau} * Bu
                    ginr = work.tile([128, TT], dtf, tag="ginr")
                    gini = work.tile([128, TT], dtf, tag="gini")
                    tmp1 = work.tile([128, TT], dtf, tag="tmp1")
                    tmp2 = work.tile([128, TT], dtf, tag="tmp2")
                    # products on DVE (PSUM-capable); combines on idle GPSIMD (SBUF-only)
                    ginr2 = work.tile([128, TT], dtf, tag="ginr2")
                    gini2 = work.tile([128, TT], dtf, tag="gini2")
                    nc.vector.tensor_mul(ginr[:, :Ti], ctg, bre[:, :Ti])
                    nc.vector.tensor_mul(tmp1[:, :Ti], stg, bim[:, :Ti])
                    nc.gpsimd.tensor_add(ginr2[:, :Ti], ginr[:, :Ti], tmp1[:, :Ti])
                    nc.vector.tensor_mul(gini[:, :Ti], ctg, bim[:, :Ti])
                    nc.vector.tensor_mul(tmp2[:, :Ti], stg, bre[:, :Ti])
                    nc.gpsimd.tensor_sub(gini2[:, :Ti], gini[:, :Ti], tmp2[:, :Ti])

                    # g_tau = a*g_{tau-1} + g_in   (real recurrence, per lane)
                    gsr = work.tile([128, TT], dtf, tag="gsr")
                    gsi = work.tile([128, TT], dtf, tag="gsi")
                    nc.vector.tensor_tensor_scan(
                        gsr[:, :Ti], at[:, g, :Ti], ginr2[:, :Ti],
                        carry[:, 2 * g : 2 * g + 1], op0=OP.mult, op1=OP.add,
                    )
                    nc.vector.tensor_copy(carry[:, 2 * g : 2 * g + 1], gsr[:, Ti - 1 : Ti])
                    nc.vector.tensor_tensor_scan(
                        gsi[:, :Ti], at[:, g, :Ti], gini2[:, :Ti],
                        carry[:, 2 * g + 1 : 2 * g + 2], op0=OP.mult, op1=OP.add,
                    )
                    nc.vector.tensor_copy(carry[:, 2 * g + 1 : 2 * g + 2], gsi[:, Ti - 1 : Ti])

                    # h = e^{+i theta tau} * g
                    hre = hkeep.tile([128, TT], dtf, tag=f"hre{g}")
                    him = hkeep.tile([128, TT], dtf, tag=f"him{g}")
                    tmp3 = work.tile([128, TT], dtf, tag="tmp3")
                    tmp4 = work.tile([128, TT], dtf, tag="tmp4")
                    hpr = work.tile([128, TT], dtf, tag="hpr")
                    hpi = work.tile([128, TT], dtf, tag="hpi")
                    nc.vector.tensor_mul(hpr[:, :Ti], ctg, gsr[:, :Ti])
                    nc.vector.tensor_mul(tmp3[:, :Ti], stg, gsi[:, :Ti])
                    nc.gpsimd.tensor_sub(hre[:, :Ti], hpr[:, :Ti], tmp3[:, :Ti])
                    nc.vector.tensor_mul(hpi[:, :Ti], ctg, gsi[:, :Ti])
                    nc.vector.tensor_mul(tmp4[:, :Ti], stg, gsr[:, :Ti])
                    nc.gpsimd.tensor_add(him[:, :Ti], hpi[:, :Ti], tmp4[:, :Ti])
                    hres.append(hre)
                    hims.append(him)

                    if i == ntiles - 1:
                        nc.sync.dma_start(out=hlast[g, :], in_=hre[:, Ti - 1 : Ti])
                        nc.sync.dma_start(out=hlast[G + g, :], in_=him[:, Ti - 1 : Ti])

                # out[t,m] = sum_h C_re[m,h] h_re - C_im[m,h] h_im + sum_m' x[t,m'] dd[m',m]
                for j in range(nj):
                    glob0 = t0 + j * 128
                    js = slice(j * 128, (j + 1) * 128)
                    ops = psum.tile([128, M], dtf, tag="ops")
                    for g in range(G):
                        nc.tensor.matmul(
                            ops, hres[g][:, js], cre_sb[:, g, :],
                            start=(g == 0), stop=False,
                        )
                        nc.tensor.matmul(
                            ops, hims[g][:, js], cimn_sb[:, g, :],
                            start=False, stop=False,
                        )
                    for k in range(G):
                        nc.tensor.matmul(
                            ops, xt[:, k, js], dd_sb[:, k, :],
                            start=False, stop=(k == G - 1),
                        )
                    if glob0 >= V:
                        osb = outp.tile([128, M], dtf, tag="osb")
                        nc.scalar.activation(osb, ops, AF.Copy)
                        nc.sync.dma_start(out=out[glob0 - V : glob0 - V + 128, :], in_=osb)
    nc.finalize()
    return nc


def _prep(state, x, start, theta_log, nu_log, gamma_log, B_re, B_im, C_re, C_im, D):
    theta = np.exp(np.asarray(theta_log, np.float64))
    r = np.exp(-np.exp(np.asarray(nu_log, np.float64)))          # [H]
    gamma = np.exp(np.asarray(gamma_log, np.float64))
    wbre = np.ascontiguousarray((np.asarray(B_re, np.float64) * gamma[:, None]).T.astype(np.float32))
    wbim = np.ascontiguousarray((np.asarray(B_im, np.float64) * gamma[:, None]).T.astype(np.float32))
    cre = np.ascontiguousarray(np.asarray(C_re, np.float32).T)
    cimn = np.ascontiguousarray((-np.asarray(C_im, np.float32)).T)
    dd = np.ascontiguousarray(np.diag(np.asarray(D, np.float32)))

    tau = np.arange(S, dtype=np.float64)
    ang = theta[:, None] * tau[None, :]
    ctab = np.ascontiguousarray(np.cos(ang).astype(np.float32))
    stab = np.ascontiguousarray(np.sin(ang).astype(np.float32))

    startb = np.asarray(start, bool)
    keep = (1.0 - startb.astype(np.float32))
    rf = r.astype(np.float32)
    x32 = np.asarray(x, np.float32)

    in_maps = []
    for c in range(NCORES):
        lo = c * SH - V
        if c == 0:
            xs = np.concatenate([np.zeros((V, M), np.float32), x32[:SH]], axis=0)
            ks = np.concatenate([np.ones(V, np.float32), keep[:SH]])
        else:
            xs = x32[lo : lo + S]
            ks = keep[lo : lo + S]
        in_maps.append(
            dict(
                xT=np.ascontiguousarray(xs.T),
                av=np.ascontiguousarray(rf[:, None] * ks[None, :]),
                ct=ctab, st=stab, wbre=wbre, wbim=wbim,
                cre=cre, cimn=cimn, dd=dd,
            )
        )
    return in_maps, theta, r, startb


def _boundary_ok(startb):
    return all(startb[c * SH - V : c * SH].any() for c in range(1, NCORES))


def kernel(state, x, start, theta_log, nu_log, gamma_log, B_re, B_im, C_re, C_im, D,
           _want_results=False):
    from concourse.bass_utils import run_bass_kernel_spmd

    in_maps, theta, r, startb = _prep(
        state, x, start, theta_log, nu_log, gamma_log, B_re, B_im, C_re, C_im, D
    )
    assert _boundary_ok(startb), "no episode reset inside an overlap window; V too small"

    if "nc" not in _CACHE:
        _CACHE["nc"] = _build_bass()
    nc = _CACHE["nc"]

    res = run_bass_kernel_spmd(nc, in_maps, list(range(NCORES))).results
    out = np.concatenate([res[c]["out"] for c in range(NCORES)], axis=0)
    hl = res[NCORES - 1]["hlast"]
    hfin = (hl[:G].reshape(H) + 1j * hl[G:].reshape(H)).astype(np.complex64)[None, :]

    # exact correction for a nonzero initial state (zero for the spec inputs):
    s0 = np.asarray(state, np.float64)[0]
    if np.any(s0 != 0):
        lam = r * np.exp(1j * theta)
        C = np.asarray(C_re, np.float64) + 1j * np.asarray(C_im, np.float64)
        fr = int(startb.argmax()) if startb.any() else T
        cum = s0.astype(np.complex128)
        for t in range(fr):
            cum = cum * lam
            out[t] += (C @ cum).real.astype(np.float32)
        if fr == T:
            hfin = (hfin[0] + cum.astype(np.complex64))[None, :]
    return hfin, out
